# revision 16
# baseline (speedup 1.0000x reference)
"""Trainium2 Bass kernel for nn_Attn_head_89412629168239.

The reference computes:
    seq_fts = x @ W1.T + b1            # [55, 8192]
    f1, f2  = seq_fts @ a1/a2 + ba     # [55]  (feeds a softmax over a
    coefs   = softmax(..., axis of size 1) = 1.0   # size-1 axis => all ones)
    out     = elu(coefs * seq_fts)[:, :, None]

Since the softmax is over a size-1 axis, coefs == 1 identically and the
f1/f2 branch is dead code.  The kernel therefore computes
    out = elu(x @ W1.T + b1)[:, :, None]
sharded column-parallel over out_sz across 8 NeuronCores (1024 columns of
W1 per core), with no collectives.  Weights are cast to bf16 on the host
(halves the HBM traffic; matmul accumulates in f32 PSUM), x is pre-
transposed on the host into the [128, ko, m] SBUF image so every DMA is
per-partition contiguous.
"""

import sys

sys.path.insert(0, "/opt/trn_rl_repo")

import ml_dtypes
import numpy as np

from concourse import bacc, bass, mybir, tile
from concourse.bass_utils import run_bass_kernel_spmd

N_NODES = 55
IN_CH = 8192
OUT_SZ = 8192
N_CORES = 8
O_SHARD = OUT_SZ // N_CORES  # 1024 output columns per core
P = 128
KT = IN_CH // P  # 64 k-tiles
NCHUNK = 512  # psum bank width in f32
N_CHUNKS = O_SHARD // NCHUNK  # 2
KO_PER_DMA = 8  # k-tiles per weight DMA chunk (2 MiB each)
N_WCHUNKS = KT // KO_PER_DMA

BF16 = mybir.dt.bfloat16
F32 = mybir.dt.float32
AF = mybir.ActivationFunctionType
ALU = mybir.AluOpType

_cache: dict = {}


def _build_nc():
    # Bacc (not plain Bass): its compile() pass splits multi-sem waits into
    # event-semaphore preludes, which walrus' 1-wait-per-instruction ISA
    # structs require.
    nc = bacc.Bacc(None)
    xs_d = nc.dram_tensor("xs", [P, KT, N_NODES], BF16, kind="ExternalInput")
    wt_d = nc.dram_tensor("wt", [P, KT, O_SHARD], BF16, kind="ExternalInput")
    # b1 packed as [bias(1024) | ones(55)] so one DMA feeds both matmul
    # operands of the K=1 bias matmul (1 sync wait, walrus limit).
    b1_d = nc.dram_tensor("b1", [1, O_SHARD + N_NODES], F32, kind="ExternalInput")
    out_d = nc.dram_tensor("out", [N_NODES, O_SHARD], F32, kind="ExternalOutput")

    with tile.TileContext(nc) as tc:
        with (
            tc.tile_pool(name="w", bufs=1) as wpool,
            tc.tile_pool(name="misc", bufs=1) as mpool,
            tc.tile_pool(name="eps", bufs=2) as epool,
            tc.tile_pool(name="psum", bufs=1, space="PSUM") as ppool,
        ):
            xs = mpool.tile([P, KT, N_NODES], BF16, name="xs_sb")
            b1 = mpool.tile([1, O_SHARD + N_NODES], F32, name="b1_sb")
            zb = mpool.tile([N_NODES, 1], F32, name="zb_sb")
            outs = mpool.tile([N_NODES, O_SHARD], F32, name="outs_sb")
            wchunks = [
                wpool.tile(
                    [P, KO_PER_DMA, O_SHARD], BF16, name=f"w{c}", tag=f"w{c}"
                )
                for c in range(N_WCHUNKS)
            ]

            nc.vector.memset(zb[:], 0.0)
            # xs/b1/out ride SWDGE (gpsimd) lanes so the 8 weight chunks own
            # the 8 HWDGE lanes 1:1 — no same-lane ordering waits anywhere
            # (walrus can't encode >1 sync wait on a DMACopy).
            nc.gpsimd.dma_start(out=xs[:], in_=xs_d[:])
            nc.gpsimd.dma_start(out=b1[:], in_=b1_d[:])
            for c in range(N_WCHUNKS):
                nc.sync.dma_start(
                    out=wchunks[c][:],
                    in_=wt_d[:, c * KO_PER_DMA : (c + 1) * KO_PER_DMA, :],
                )

            psums = [
                ppool.tile([N_NODES, NCHUNK], F32, name=f"ps{n}", tag=f"ps{n}")
                for n in range(N_CHUNKS)
            ]
            for ko in range(KT):
                w = wchunks[ko // KO_PER_DMA]
                ki = ko % KO_PER_DMA
                for n in range(N_CHUNKS):
                    nc.tensor.matmul(
                        psums[n][:, :],
                        xs[:, ko, :],
                        w[:, ki, n * NCHUNK : (n + 1) * NCHUNK],
                        start=(ko == 0),
                        stop=False,
                    )
            # bias via a K=1 matmul: psum[m, n] += ones[m] * b1[n]
            for n in range(N_CHUNKS):
                nc.tensor.matmul(
                    psums[n][:, :],
                    b1[:, O_SHARD : O_SHARD + N_NODES],
                    b1[:, n * NCHUNK : (n + 1) * NCHUNK],
                    start=False,
                    stop=True,
                )
            # elu(v) = max(v, 0) + (exp(min(v, 0)) - 1)
            # Staged engine-by-engine so no single instruction needs more
            # than one cross-engine semaphore wait (walrus can't encode >1
            # on the DVE tensor-scalar struct).
            ts_ = [
                epool.tile([N_NODES, NCHUNK], F32, name=f"t{n}", tag=f"t{n}")
                for n in range(N_CHUNKS)
            ]
            rs_ = [
                epool.tile([N_NODES, NCHUNK], F32, name=f"r{n}", tag=f"r{n}")
                for n in range(N_CHUNKS)
            ]
            es_ = [
                epool.tile([N_NODES, NCHUNK], F32, name=f"e{n}", tag=f"e{n}")
                for n in range(N_CHUNKS)
            ]
            for n in range(N_CHUNKS):
                nc.vector.tensor_scalar_min(ts_[n][:], psums[n][:], 0.0)
            # exp(min) and relu both on ACT so the final combine has a
            # single-engine (ACT) input set => one sync wait on DVE.
            for n in range(N_CHUNKS):
                nc.scalar.activation(es_[n][:], ts_[n][:], AF.Exp, bias=zb[:, 0:1])
                nc.scalar.activation(rs_[n][:], psums[n][:], AF.Relu, bias=zb[:, 0:1])
            for n in range(N_CHUNKS):
                nc.vector.scalar_tensor_tensor(
                    outs[:, n * NCHUNK : (n + 1) * NCHUNK],
                    es_[n][:],
                    -1.0,
                    rs_[n][:],
                    ALU.add,
                    ALU.add,
                )
            nc.gpsimd.dma_start(out=out_d[:], in_=outs[:])
    # run the bacc passes (event-semaphore generation, register allocation,
    # nop fusion) — run_bass_via_pjrt does not finalize a prebuilt nc.
    nc.compile()
    return nc


def _prep_inputs(x, W1, b1):
    """Host-side shard + layout prep.

    Returns per-core in_maps.  Layouts are the exact SBUF images so every
    DMA is per-partition contiguous:
      xs[p, ko, m]     = x[m, ko*128 + p]          (bf16, replicated)
      wt[p, ko, n]     = W1[c*1024 + n, ko*128+p]  (bf16, per-core shard)
      b1[0, n]         = b1[c*1024 + n]            (f32)
    """
    x = np.asarray(x, dtype=np.float32)
    W1 = np.asarray(W1, dtype=np.float32)
    b1 = np.asarray(b1, dtype=np.float32)

    xs = np.ascontiguousarray(
        x.T.reshape(KT, P, N_NODES).transpose(1, 0, 2)
    ).astype(ml_dtypes.bfloat16)

    in_maps = []
    for c in range(N_CORES):
        Ws = W1[c * O_SHARD : (c + 1) * O_SHARD]  # [1024, 8192]
        wt = np.ascontiguousarray(
            Ws.T.reshape(KT, P, O_SHARD).transpose(1, 0, 2)
        ).astype(ml_dtypes.bfloat16)
        b1_packed = np.concatenate(
            [b1[c * O_SHARD : (c + 1) * O_SHARD], np.ones(N_NODES, np.float32)]
        )[None, :]
        in_maps.append(
            {
                "xs": xs,
                "wt": wt,
                "b1": np.ascontiguousarray(b1_packed),
            }
        )
    return in_maps


def _run(inputs: dict, trace: bool = False):
    """Run the kernel; returns (full_output, BassKernelResults)."""
    if "nc" not in _cache:
        _cache["nc"] = _build_nc()
    nc = _cache["nc"]
    in_maps = _prep_inputs(inputs["x"], inputs["W1"], inputs["b1"])
    res = run_bass_kernel_spmd(
        nc, in_maps, core_ids=list(range(N_CORES)), trace=trace
    )
    shards = [np.asarray(res.results[i]["out"]) for i in range(N_CORES)]
    full = np.concatenate(shards, axis=1).astype(np.float32)  # [55, 8192]
    return full[:, :, None], res


def kernel(**inputs) -> np.ndarray:
    out, _ = _run(inputs, trace=False)
    return out


# revision 17
# speedup vs baseline: 1.0068x; 1.0068x over previous
"""Trainium2 Bass kernel for nn_Attn_head_89412629168239.

The reference computes:
    seq_fts = x @ W1.T + b1            # [55, 8192]
    f1, f2  = seq_fts @ a1/a2 + ba     # [55]  (feeds a softmax over a
    coefs   = softmax(..., axis of size 1) = 1.0   # size-1 axis => all ones)
    out     = elu(coefs * seq_fts)[:, :, None]

Since the softmax is over a size-1 axis, coefs == 1 identically and the
f1/f2 branch is dead code.  The kernel therefore computes
    out = elu(x @ W1.T + b1)[:, :, None]
sharded column-parallel over out_sz across 8 NeuronCores (1024 columns of
W1 per core), with no collectives.  Weights are cast to bf16 on the host
(halves the HBM traffic; matmul accumulates in f32 PSUM), x is pre-
transposed on the host into the [128, ko, m] SBUF image so every DMA is
per-partition contiguous.
"""

import sys

sys.path.insert(0, "/opt/trn_rl_repo")

import ml_dtypes
import numpy as np

from concourse import bacc, bass, mybir, tile
from concourse.bass_utils import run_bass_kernel_spmd

N_NODES = 55
IN_CH = 8192
OUT_SZ = 8192
N_CORES = 8
O_SHARD = OUT_SZ // N_CORES  # 1024 output columns per core
P = 128
KT = IN_CH // P  # 64 k-tiles
NCHUNK = 512  # psum bank width in f32
N_CHUNKS = O_SHARD // NCHUNK  # 2
KO_PER_DMA = 8  # k-tiles per weight DMA chunk (2 MiB each)
N_WCHUNKS = KT // KO_PER_DMA

BF16 = mybir.dt.bfloat16
F32 = mybir.dt.float32
AF = mybir.ActivationFunctionType
ALU = mybir.AluOpType

_cache: dict = {}


def _build_nc():
    # Bacc (not plain Bass): its compile() pass splits multi-sem waits into
    # event-semaphore preludes, which walrus' 1-wait-per-instruction ISA
    # structs require.
    nc = bacc.Bacc(None)
    xs_d = nc.dram_tensor("xs", [P, KT, N_NODES], BF16, kind="ExternalInput")
    wt_d = nc.dram_tensor("wt", [P, KT, O_SHARD], BF16, kind="ExternalInput")
    # b1 packed as [bias(1024) | ones(55)] so one DMA feeds both matmul
    # operands of the K=1 bias matmul (1 sync wait, walrus limit).
    b1_d = nc.dram_tensor("b1", [1, O_SHARD + N_NODES], F32, kind="ExternalInput")
    out_d = nc.dram_tensor("out", [N_NODES, O_SHARD], F32, kind="ExternalOutput")

    with tile.TileContext(nc) as tc:
        with (
            tc.tile_pool(name="w", bufs=1) as wpool,
            tc.tile_pool(name="misc", bufs=1) as mpool,
            tc.tile_pool(name="eps", bufs=2) as epool,
            tc.tile_pool(name="psum", bufs=1, space="PSUM") as ppool,
        ):
            xs = mpool.tile([P, KT, N_NODES], BF16, name="xs_sb")
            b1 = mpool.tile([1, O_SHARD + N_NODES], F32, name="b1_sb")
            zb = mpool.tile([N_NODES, 1], F32, name="zb_sb")
            outs = mpool.tile([N_NODES, O_SHARD], F32, name="outs_sb")
            wchunks = [
                wpool.tile(
                    [P, KO_PER_DMA, O_SHARD], BF16, name=f"w{c}", tag=f"w{c}"
                )
                for c in range(N_WCHUNKS)
            ]

            nc.vector.memset(zb[:], 0.0)
            # xs/b1/out ride SWDGE (gpsimd) lanes so the 8 weight chunks own
            # the 8 HWDGE lanes 1:1 — no same-lane ordering waits anywhere
            # (walrus can't encode >1 sync wait on a DMACopy).
            nc.gpsimd.dma_start(out=xs[:], in_=xs_d[:])
            nc.gpsimd.dma_start(out=b1[:], in_=b1_d[:])
            for c in range(N_WCHUNKS):
                nc.sync.dma_start(
                    out=wchunks[c][:],
                    in_=wt_d[:, c * KO_PER_DMA : (c + 1) * KO_PER_DMA, :],
                )

            psums = [
                ppool.tile([N_NODES, NCHUNK], F32, name=f"ps{n}", tag=f"ps{n}")
                for n in range(N_CHUNKS)
            ]
            for ko in range(KT):
                w = wchunks[ko // KO_PER_DMA]
                ki = ko % KO_PER_DMA
                for n in range(N_CHUNKS):
                    nc.tensor.matmul(
                        psums[n][:, :],
                        xs[:, ko, :],
                        w[:, ki, n * NCHUNK : (n + 1) * NCHUNK],
                        start=(ko == 0),
                        stop=False,
                    )
            # bias via a K=1 matmul: psum[m, n] += ones[m] * b1[n]
            for n in range(N_CHUNKS):
                nc.tensor.matmul(
                    psums[n][:, :],
                    b1[:, O_SHARD : O_SHARD + N_NODES],
                    b1[:, n * NCHUNK : (n + 1) * NCHUNK],
                    start=False,
                    stop=True,
                )
            # elu(v) = max(v, 0) + (exp(min(v, 0)) - 1)
            # Staged engine-by-engine so no single instruction needs more
            # than one cross-engine semaphore wait (walrus can't encode >1
            # on the DVE tensor-scalar struct).
            ts_ = [
                epool.tile([N_NODES, NCHUNK], F32, name=f"t{n}", tag=f"t{n}")
                for n in range(N_CHUNKS)
            ]
            rs_ = [
                epool.tile([N_NODES, NCHUNK], F32, name=f"r{n}", tag=f"r{n}")
                for n in range(N_CHUNKS)
            ]
            es_ = [
                epool.tile([N_NODES, NCHUNK], F32, name=f"e{n}", tag=f"e{n}")
                for n in range(N_CHUNKS)
            ]
            for n in range(N_CHUNKS):
                nc.vector.tensor_scalar_min(ts_[n][:], psums[n][:], 0.0)
            # exp(min) and relu both on ACT so the final combine has a
            # single-engine (ACT) input set => one sync wait on DVE.
            for n in range(N_CHUNKS):
                nc.scalar.activation(es_[n][:], ts_[n][:], AF.Exp, bias=zb[:, 0:1])
                nc.scalar.activation(rs_[n][:], psums[n][:], AF.Relu, bias=zb[:, 0:1])
            for n in range(N_CHUNKS):
                nc.vector.scalar_tensor_tensor(
                    outs[:, n * NCHUNK : (n + 1) * NCHUNK],
                    es_[n][:],
                    -1.0,
                    rs_[n][:],
                    ALU.add,
                    ALU.add,
                )
            nc.gpsimd.dma_start(out=out_d[:], in_=outs[:])
    # run the bacc passes (event-semaphore generation, register allocation,
    # nop fusion) — run_bass_via_pjrt does not finalize a prebuilt nc.
    nc.compile()
    return nc


def _prep_inputs(x, W1, b1):
    """Host-side shard + layout prep.

    Returns per-core in_maps.  Layouts are the exact SBUF images so every
    DMA is per-partition contiguous:
      xs[p, ko, m]     = x[m, ko*128 + p]          (bf16, replicated)
      wt[p, ko, n]     = W1[c*1024 + n, ko*128+p]  (bf16, per-core shard)
      b1[0, n]         = b1[c*1024 + n]            (f32)
    """
    x = np.asarray(x, dtype=np.float32)
    W1 = np.asarray(W1, dtype=np.float32)
    b1 = np.asarray(b1, dtype=np.float32)

    xs = np.ascontiguousarray(
        x.T.reshape(KT, P, N_NODES).transpose(1, 0, 2)
    ).astype(ml_dtypes.bfloat16)

    in_maps = []
    for c in range(N_CORES):
        Ws = W1[c * O_SHARD : (c + 1) * O_SHARD]  # [1024, 8192]
        wt = np.ascontiguousarray(
            Ws.T.reshape(KT, P, O_SHARD).transpose(1, 0, 2)
        ).astype(ml_dtypes.bfloat16)
        b1_packed = np.concatenate(
            [b1[c * O_SHARD : (c + 1) * O_SHARD], np.ones(N_NODES, np.float32)]
        )[None, :]
        in_maps.append(
            {
                "xs": xs,
                "wt": wt,
                "b1": np.ascontiguousarray(b1_packed),
            }
        )
    return in_maps


def _run(inputs: dict, trace: bool = False, tmpdir: str | None = None):
    """Run the kernel; returns (full_output, BassKernelResults)."""
    if "nc" not in _cache:
        _cache["nc"] = _build_nc()
    nc = _cache["nc"]
    in_maps = _prep_inputs(inputs["x"], inputs["W1"], inputs["b1"])
    res = run_bass_kernel_spmd(
        nc, in_maps, core_ids=list(range(N_CORES)), trace=trace, tmpdir=tmpdir
    )
    shards = [np.asarray(res.results[i]["out"]) for i in range(N_CORES)]
    full = np.concatenate(shards, axis=1).astype(np.float32)  # [55, 8192]
    return full[:, :, None], res


def kernel(**inputs) -> np.ndarray:
    out, _ = _run(inputs, trace=False)
    return out


# revision 22
# speedup vs baseline: 1.1110x; 1.1034x over previous
"""Trainium2 Bass kernel for nn_Attn_head_89412629168239.

The reference computes:
    seq_fts = x @ W1.T + b1            # [55, 8192]
    f1, f2  = seq_fts @ a1/a2 + ba     # [55]  (feeds a softmax over a
    coefs   = softmax(..., axis of size 1) = 1.0   # size-1 axis => all ones)
    out     = elu(coefs * seq_fts)[:, :, None]

Since the softmax is over a size-1 axis, coefs == 1 identically and the
f1/f2 branch is dead code.  The kernel therefore computes
    out = elu(x @ W1.T + b1)[:, :, None]
sharded column-parallel over out_sz across 8 NeuronCores (1024 columns of
W1 per core), with no collectives.  Weights are cast to bf16 on the host
(halves the HBM traffic; matmul accumulates in f32 PSUM), x is pre-
transposed on the host into the [128, ko, m] SBUF image so every DMA is
per-partition contiguous.
"""

import sys

sys.path.insert(0, "/opt/trn_rl_repo")

import ml_dtypes
import numpy as np

from concourse import bacc, bass, mybir, tile
from concourse import bass_utils as _bass_utils
from concourse.bass_utils import run_bass_kernel_spmd



N_NODES = 55
IN_CH = 8192
OUT_SZ = 8192
N_CORES = 8
O_SHARD = OUT_SZ // N_CORES  # 1024 output columns per core
P = 128
KT = IN_CH // P  # 64 k-tiles
NCHUNK = 512  # psum bank width in f32
N_CHUNKS = O_SHARD // NCHUNK  # 2
KO_PER_DMA = 8  # k-tiles per weight DMA chunk (2 MiB each)
N_WCHUNKS = KT // KO_PER_DMA

BF16 = mybir.dt.bfloat16
F32 = mybir.dt.float32
AF = mybir.ActivationFunctionType
ALU = mybir.AluOpType

_cache: dict = {}


def _build_nc():
    # Bacc (not plain Bass): its compile() pass splits multi-sem waits into
    # event-semaphore preludes, which walrus' 1-wait-per-instruction ISA
    # structs require.
    nc = bacc.Bacc(None)
    xs_d = nc.dram_tensor("xs", [P, KT, N_NODES], BF16, kind="ExternalInput")
    wt_d = nc.dram_tensor("wt", [P, KT, O_SHARD], BF16, kind="ExternalInput")
    # b1 packed as [bias(1024) | ones(55)] so one DMA feeds both matmul
    # operands of the K=1 bias matmul (1 sync wait, walrus limit).
    b1_d = nc.dram_tensor("b1", [1, O_SHARD + N_NODES], F32, kind="ExternalInput")
    out_d = nc.dram_tensor("out", [N_NODES, O_SHARD], F32, kind="ExternalOutput")

    with tile.TileContext(nc) as tc:
        with (
            tc.tile_pool(name="w", bufs=1) as wpool,
            tc.tile_pool(name="misc", bufs=1) as mpool,
            tc.tile_pool(name="eps", bufs=2) as epool,
            tc.tile_pool(name="psum", bufs=1, space="PSUM") as ppool,
        ):
            xs = mpool.tile([P, KT, N_NODES], BF16, name="xs_sb")
            b1 = mpool.tile([1, O_SHARD + N_NODES], F32, name="b1_sb")
            zb = mpool.tile([N_NODES, 1], F32, name="zb_sb")
            outs = mpool.tile([N_NODES, O_SHARD], F32, name="outs_sb")
            wchunks = [
                wpool.tile(
                    [P, KO_PER_DMA, O_SHARD], BF16, name=f"w{c}", tag=f"w{c}"
                )
                for c in range(N_WCHUNKS)
            ]

            nc.vector.memset(zb[:], 0.0)
            # xs/b1 on the ACT HWDGE ring (fast first-byte, issue overlaps
            # the weight-DMA issue on the SP ring); the weight chunks go
            # FIFO on the SP ring so their completions stagger and matmuls
            # can chase the data.  The output DMA rides SWDGE (gpsimd) so
            # it never shares a lane sem with an input DMA (walrus can't
            # encode >1 sync wait on a DMACopy).
            nc.scalar.dma_start(out=xs[:], in_=xs_d[:])
            nc.scalar.dma_start(out=b1[:], in_=b1_d[:])
            for c in range(N_WCHUNKS):
                nc.sync.dma_start(
                    out=wchunks[c][:],
                    in_=wt_d[:, c * KO_PER_DMA : (c + 1) * KO_PER_DMA, :],
                )

            psums = [
                ppool.tile([N_NODES, NCHUNK], F32, name=f"ps{n}", tag=f"ps{n}")
                for n in range(N_CHUNKS)
            ]
            for ko in range(KT):
                w = wchunks[ko // KO_PER_DMA]
                ki = ko % KO_PER_DMA
                for n in range(N_CHUNKS):
                    nc.tensor.matmul(
                        psums[n][:, :],
                        xs[:, ko, :],
                        w[:, ki, n * NCHUNK : (n + 1) * NCHUNK],
                        start=(ko == 0),
                        stop=False,
                    )
            # bias via a K=1 matmul: psum[m, n] += ones[m] * b1[n]
            for n in range(N_CHUNKS):
                nc.tensor.matmul(
                    psums[n][:, :],
                    b1[:, O_SHARD : O_SHARD + N_NODES],
                    b1[:, n * NCHUNK : (n + 1) * NCHUNK],
                    start=False,
                    stop=True,
                )
            # elu(v) = max(v, 0) + (exp(min(v, 0)) - 1)
            # Staged engine-by-engine so no single instruction needs more
            # than one cross-engine semaphore wait (walrus can't encode >1
            # on the DVE tensor-scalar struct).
            ts_ = [
                epool.tile([N_NODES, NCHUNK], F32, name=f"t{n}", tag=f"t{n}")
                for n in range(N_CHUNKS)
            ]
            rs_ = [
                epool.tile([N_NODES, NCHUNK], F32, name=f"r{n}", tag=f"r{n}")
                for n in range(N_CHUNKS)
            ]
            es_ = [
                epool.tile([N_NODES, NCHUNK], F32, name=f"e{n}", tag=f"e{n}")
                for n in range(N_CHUNKS)
            ]
            for n in range(N_CHUNKS):
                nc.vector.tensor_scalar_min(ts_[n][:], psums[n][:], 0.0)
            # exp(min) and relu both on ACT so the final combine has a
            # single-engine (ACT) input set => one sync wait on DVE.
            for n in range(N_CHUNKS):
                nc.scalar.activation(es_[n][:], ts_[n][:], AF.Exp, bias=zb[:, 0:1])
                nc.scalar.activation(rs_[n][:], psums[n][:], AF.Relu, bias=zb[:, 0:1])
            for n in range(N_CHUNKS):
                nc.vector.scalar_tensor_tensor(
                    outs[:, n * NCHUNK : (n + 1) * NCHUNK],
                    es_[n][:],
                    -1.0,
                    rs_[n][:],
                    ALU.add,
                    ALU.add,
                )
            nc.gpsimd.dma_start(out=out_d[:], in_=outs[:])
    _dedupe_ldweights(nc)
    # run the bacc passes (event-semaphore generation, register allocation,
    # nop fusion) — run_bass_via_pjrt does not finalize a prebuilt nc.
    nc.compile()
    return nc


def _dedupe_ldweights(nc):
    """Drop InstLdweights that reload the exact weights already resident.

    tile_legalize splits every bf16 matmul into LDWEIGHTS + MATMUL; our two
    n-chunk matmuls per k-tile share one stationary operand, so half the
    loads are redundant.  Removing them lets the second matmul pipeline
    directly behind the first (PE fill/drain overlap) instead of
    serializing on a weight reload.  Only waits/update-free loads with an
    identical physical AP are dropped; any f32 (self-loading) matmul
    invalidates the tracked weight state.
    """
    removed = 0
    for bb in nc.m.functions[0].blocks:
        il = bb.instructions
        last_key = None
        keep = []
        for ins in il:
            tn = type(ins).__name__
            if tn == "InstLdweights":
                a = ins.ins[0]
                key = (a.memref, a.offset, str(a.ap), str(a.dtype))
                si = ins.sync_info
                clean = si is None or (not si.on_wait and not si.on_update)
                if key == last_key and clean:
                    nc.inst_map.pop(ins.name, None)
                    removed += 1
                    continue
                last_key = key
            elif tn == "InstMatmult":
                stat = ins.ins[1] if len(ins.ins) > 1 else None
                if stat is not None and "float32" in str(
                    getattr(stat, "dtype", "")
                ):
                    last_key = None
            keep.append(ins)
        if removed:
            il[:] = keep
    return removed


def _prep_inputs(x, W1, b1):
    """Host-side shard + layout prep.

    Returns per-core in_maps.  Layouts are the exact SBUF images so every
    DMA is per-partition contiguous:
      xs[p, ko, m]     = x[m, ko*128 + p]          (bf16, replicated)
      wt[p, ko, n]     = W1[c*1024 + n, ko*128+p]  (bf16, per-core shard)
      b1[0, n]         = b1[c*1024 + n]            (f32)
    """
    x = np.asarray(x, dtype=np.float32)
    W1 = np.asarray(W1, dtype=np.float32)
    b1 = np.asarray(b1, dtype=np.float32)

    xs = np.ascontiguousarray(
        x.T.reshape(KT, P, N_NODES).transpose(1, 0, 2)
    ).astype(ml_dtypes.bfloat16)

    in_maps = []
    for c in range(N_CORES):
        Ws = W1[c * O_SHARD : (c + 1) * O_SHARD]  # [1024, 8192]
        wt = np.ascontiguousarray(
            Ws.T.reshape(KT, P, O_SHARD).transpose(1, 0, 2)
        ).astype(ml_dtypes.bfloat16)
        b1_packed = np.concatenate(
            [b1[c * O_SHARD : (c + 1) * O_SHARD], np.ones(N_NODES, np.float32)]
        )[None, :]
        in_maps.append(
            {
                "xs": xs,
                "wt": wt,
                "b1": np.ascontiguousarray(b1_packed),
            }
        )
    return in_maps


def _run(inputs: dict, trace: bool = False, tmpdir: str | None = None):
    """Run the kernel; returns (full_output, BassKernelResults)."""
    if "nc" not in _cache:
        _cache["nc"] = _build_nc()
    nc = _cache["nc"]
    in_maps = _prep_inputs(inputs["x"], inputs["W1"], inputs["b1"])
    res = run_bass_kernel_spmd(
        nc, in_maps, core_ids=list(range(N_CORES)), trace=trace, tmpdir=tmpdir
    )
    shards = [np.asarray(res.results[i]["out"]) for i in range(N_CORES)]
    full = np.concatenate(shards, axis=1).astype(np.float32)  # [55, 8192]
    return full[:, :, None], res


def kernel(**inputs) -> np.ndarray:
    out, _ = _run(inputs, trace=False)
    return out
